# revision 18
# baseline (speedup 1.0000x reference)
"""Conditional_Embedding_Contrastive_loss Trainium2 kernel (8 cores).

Full-input contract: kernel(**inputs) takes the complete tensors and
returns the scalar loss. End-to-end wall time is dominated by the axon
host->device tunnel (~60-110 MB/s effective, ~50ms sync RTT) and by
host-side marshalling, so the implementation minimizes bytes moved and
round trips:

  1. Each core receives ONLY its own int4-packed shard of the
     row-normalized embedding matrix (x-hat * 16 quantized to step 0.25,
     two columns per byte; 256 KB/core). The full operand is assembled
     on-device with a DRAM AllGather over NeuronLink and unpacked to fp8
     (the 17 quantized levels are exactly fp8-representable, so int4
     packing costs no extra precision vs the quantization itself).
  2. Row norms, the anchor cosine term p_i, and the analytic diagonal
     corrections are computed on the host (one fused XLA-CPU jit) and
     folded into a tiny per-row pair (cnum, cden):
         logq_i = ln(S_msk_i + cnum_i) - ln(S_all_i + cden_i)
     with cnum_i = p_i - exp(1/T)*m_ii, cden_i = p_i - exp(1/T), where
     S_all/S_msk are full-row sums of exp(sim/T) (resp. masked by
     cls_mask[labels_i]) including the diagonal.
  3. The 0/1 mask rows are bit-packed on the host (plane-major: byte k,
     bit b <-> column b*(N/8)+k) to 256 KB/core and unpacked on-device
     with shift+and DVE ops. The (cnum, cden) f32 pair rides as 8
     trailing bytes per mask row (read on device via AP bitcast), so the
     whole call issues only two h2d arrays + one d2h fetch.
  4. The shard_map jit is built once per process and cached; prep jit
     outputs are materialized before device_put so the h2d of each
     array overlaps the compute of the next (device_put of a lazy cpu
     array would block).

Device pipeline per core (R = N/8 = 512 rows, P = 128):
  - DRAM AllGather: xp [D, R/2] u8 -> xg [8*D, R/2].
  - int4 unpack: (b&15), (b>>4)&15 -> fp8 via TSP mult/sub (u8 in, fp8
    out) into xt_sb [128, D/128, N] fp8; own shard likewise.
  - per row-block b (4) and j-tile (1024 cols): PE fp8 matmul (8
    k-chunks, 2x512-wide) -> PSUM; ACT exp(scale=1/(T*256)) PSUM->SBUF
    with accum_out = unmasked row-sum; DVE scalar_tensor_tensor e*mask
    with accum_out = masked row-sum.
  - tail per block: two Ln on ACT, subtract, DMA out logq [NB,P,1].
Host: loss = -mean(logq).
"""

import sys

for _p in ("/opt/trn_rl_repo",):
    if _p not in sys.path:
        sys.path.insert(0, _p)

import numpy as np
import ml_dtypes

P = 128          # SBUF partitions
JW = 512         # PE moving free-dim max
EPS = 1e-8

_CACHE = {}

XS = 16.0        # pre-scale: matmul yields XS^2 * sim, folded out in the exp
QL = 0.25        # int4 quant step of (x-hat * XS); levels (v-8)*QL


def build_kernel(N, D, R, inv_T, n_cores=8, shared_cc_out=True,
                 mpsum_bufs=3, work_bufs=2, mask_bufs=2, stage_bufs=3):
    """Build the SPMD Bass program for one core owning R rows of N total."""
    import concourse.bass as bass
    import concourse.mybir as mybir
    import concourse.tile as tile
    from concourse import bacc

    f32 = mybir.dt.float32
    bf16 = mybir.dt.bfloat16
    fp8 = mybir.dt.float8e4
    u8 = mybir.dt.uint8
    exp_scale = float(inv_T / (XS * XS))
    Exp = mybir.ActivationFunctionType.Exp
    Ln = mybir.ActivationFunctionType.Ln
    mult = mybir.AluOpType.mult
    sub = mybir.AluOpType.subtract
    shr = mybir.AluOpType.logical_shift_right
    band = mybir.AluOpType.bitwise_and
    X = mybir.AxisListType.X

    KC = D // P        # contraction chunks of 128
    NB = R // P        # own row blocks
    RH = R // 2        # packed bytes per row-shard line (2 cols/byte)
    JT = min(1024, N)  # j-tile width (2 PSUM banks of fp32)
    JC = N // JT       # j tiles per row block
    NH = JT // JW      # matmuls per j-tile per k-chunk
    NPB = N // 8       # packed-mask bytes per row (one bit-plane's width)

    nc = bacc.Bacc(
        "TRN2", target_bir_lowering=False, debug=False, num_devices=n_cores)
    xp_d = nc.declare_dram_parameter("xp", [D, RH], u8, isOutput=False)
    # mask rows + 8 trailing bytes per row = (cnum, cden) f32 pair
    mpk_d = nc.declare_dram_parameter("mpk", [R, NPB + 8], u8, isOutput=False)
    out_d = nc.declare_dram_parameter("logq", [NB, P, 1], f32, isOutput=True)

    with tile.TileContext(nc) as tc:
        with (
            tc.tile_pool(name="big", bufs=1) as big,
            tc.tile_pool(name="stage", bufs=stage_bufs) as stagep,
            tc.tile_pool(name="mask", bufs=mask_bufs) as maskp,
            tc.tile_pool(name="work", bufs=work_bufs) as workp,
            tc.tile_pool(name="stats", bufs=1) as statsp,
            tc.tile_pool(name="tiny", bufs=2) as tinyp,
            tc.tile_pool(name="dram", bufs=1, space="DRAM") as dramp,
            tc.tile_pool(name="mpsum", bufs=mpsum_bufs, space="PSUM") as mpsum,
        ):
            xt_sb = big.tile([P, KC, N], fp8)
            xst_sb = big.tile([P, KC, R], fp8)
            mpk_sb = big.tile([P, NB, NPB], u8)
            cv_sb = statsp.tile([P, NB, 8], u8)
            accA = statsp.tile([P, NB, JC], f32)
            accM = statsp.tile([P, NB, JC], f32)
            logq = statsp.tile([P, NB], f32)

            xin_b = dramp.tile([D, RH], u8)
            xg_b = dramp.tile(
                [n_cores * D, RH], u8,
                addr_space="Shared" if shared_cc_out else "Local")

            # ---- collective: own packed shard -> full gathered matrix ----
            nc.sync.dma_start(xin_b[:], xp_d[:, :])
            nc.gpsimd.collective_compute(
                "AllGather", mybir.AluOpType.bypass,
                replica_groups=[list(range(n_cores))],
                ins=[xin_b.opt()], outs=[xg_b.opt()])

            # ---- input DMAs that don't depend on the collective ----
            for b in range(NB):
                nc.sync.dma_start(
                    mpk_sb[:, b, :], mpk_d[b * P:(b + 1) * P, 0:NPB])
                nc.sync.dma_start(
                    cv_sb[:, b, :], mpk_d[b * P:(b + 1) * P, NPB:NPB + 8])

            # Pre-place the combined ln+exp activation table (a table switch
            # costs ~2.7us on the scalar engine).
            ACT_SET_LN_EXP = 6  # natural_log_exp_and_others (gen3 act_info)
            nc.scalar.add_instruction(mybir.InstLoadActFuncSet(
                name=nc.get_next_instruction_name(),
                act_func_set_id=ACT_SET_LN_EXP, ins=[], outs=[]))

            def unpack4(dst_lo, dst_hi, src_u8):
                """int4 pair -> two fp8 column groups: (v-8)*QL each."""
                lo = stagep.tile([P, RH], u8, tag="lo", name="lo")
                hi = stagep.tile([P, RH], u8, tag="hi", name="hi")
                nc.vector.tensor_scalar(lo, src_u8, 15, None, op0=band)
                nc.vector.tensor_scalar(hi, src_u8, 4, 15, op0=shr, op1=band)
                # arith TSP casts u8 -> fp8: out = v*QL - 8*QL
                nc.vector.tensor_scalar(
                    dst_lo, lo, float(QL), float(8 * QL), op0=mult, op1=sub)
                nc.vector.tensor_scalar(
                    dst_hi, hi, float(QL), float(8 * QL), op0=mult, op1=sub)

            # ---- own shard unpack (param direct; overlaps collective) ----
            for c in range(KC):
                pko = stagep.tile([P, RH], u8, tag="pk", name="pko")
                nc.sync.dma_start(pko, xp_d[c * P:(c + 1) * P, :])
                unpack4(xst_sb[:, c, 0:RH], xst_sb[:, c, RH:R], pko)

            # ---- gathered shards -> SBUF (unpacked) ----
            for k in range(n_cores):
                for c in range(KC):
                    pkg = stagep.tile([P, RH], u8, tag="pk", name="pkg")
                    nc.sync.dma_start(
                        pkg, xg_b[k * D + c * P: k * D + (c + 1) * P, :])
                    unpack4(xt_sb[:, c, k * R: k * R + RH],
                            xt_sb[:, c, k * R + RH: (k + 1) * R], pkg)

            # ---- main loop ----
            for b in range(NB):
                # unpack this block's mask rows: bit-plane pl covers columns
                # [pl*NPB, (pl+1)*NPB). bitVec TSP ops can't cast dtypes, so
                # (>>pl)&1 stays u8->u8 and a mult-by-1 TSP does u8->bf16.
                m_sb = maskp.tile([P, N], bf16, tag="m", name="m_sb")
                for pl in range(8):
                    msh = maskp.tile([P, NPB], u8, tag="msh", name="msh")
                    nc.vector.tensor_scalar(
                        msh, mpk_sb[:, b, :], pl, 1, op0=shr, op1=band)
                    nc.vector.tensor_scalar_mul(
                        m_sb[:, pl * NPB:(pl + 1) * NPB], msh, 1)
                for jq in range(JC):
                    ps = mpsum.tile([P, JT], f32, tag="ps", name="ps")
                    for c in range(KC):
                        for h in range(NH):
                            nc.tensor.matmul(
                                ps[:, h * JW:(h + 1) * JW],
                                xst_sb[:, c, b * P:(b + 1) * P],
                                xt_sb[:, c, jq * JT + h * JW:
                                      jq * JT + (h + 1) * JW],
                                start=(c == 0), stop=(c == KC - 1))
                    e = workp.tile([P, JT], f32, tag="e", name="e")
                    nc.scalar.activation(
                        e, ps[:], Exp, scale=exp_scale,
                        accum_out=accA[:, b, jq:jq + 1])
                    junk = workp.tile([P, JT], f32, tag="junk", name="junk")
                    nc.vector.scalar_tensor_tensor(
                        out=junk, in0=e, scalar=1.0,
                        in1=m_sb[:, jq * JT:(jq + 1) * JT],
                        op0=mult, op1=mult,
                        accum_out=accM[:, b, jq:jq + 1])
                # tail: logq for block b
                sA = tinyp.tile([P, 1], f32, tag="sA")
                sM = tinyp.tile([P, 1], f32, tag="sM")
                nc.vector.reduce_sum(sA, accA[:, b, :], axis=X)
                nc.vector.reduce_sum(sM, accM[:, b, :], axis=X)
                num = tinyp.tile([P, 1], f32, tag="num")
                den = tinyp.tile([P, 1], f32, tag="den")
                cv = cv_sb[:, b, :].bitcast(f32)
                nc.vector.tensor_add(num, sM, cv[:, 0:1])
                nc.vector.tensor_add(den, sA, cv[:, 1:2])
                lnn = tinyp.tile([P, 1], f32, tag="lnn")
                lnd = tinyp.tile([P, 1], f32, tag="lnd")
                nc.scalar.activation(lnn, num, Ln)
                nc.scalar.activation(lnd, den, Ln)
                nc.vector.tensor_sub(logq[:, b:b + 1], lnn, lnd)
                nc.sync.dma_start(out_d[b], logq[:, b:b + 1])

    nc.compile()
    return nc


class _Runner:
    """shard_map jit built once; warm calls skip trace/lower/compile."""

    def __init__(self, nc, n_cores):
        import jax
        from jax.sharding import Mesh, PartitionSpec
        try:
            from jax.experimental.shard_map import shard_map
        except ImportError:
            from jax import shard_map
        import concourse.mybir as mybir
        from concourse import bass2jax

        bass2jax.install_neuronx_cc_hook()
        self.n_cores = n_cores
        self.in_names = []
        self.out_names = []
        out_avals = []
        self.zero_outs = []
        partition_name = (nc.partition_id_tensor.name
                          if nc.partition_id_tensor else None)
        for alloc in nc.m.functions[0].allocations:
            if not isinstance(alloc, mybir.MemoryLocationSet):
                continue
            name = alloc.memorylocations[0].name
            if alloc.kind == "ExternalInput":
                if name != partition_name:
                    self.in_names.append(name)
            elif alloc.kind == "ExternalOutput":
                shape = tuple(alloc.tensor_shape)
                dtype = mybir.dt.np(alloc.dtype)
                out_avals.append(jax.core.ShapedArray(shape, dtype))
                self.out_names.append(name)
                self.zero_outs.append(np.zeros(
                    (n_cores * shape[0],) + shape[1:], dtype))
        self.n_params = len(self.in_names)
        all_in = list(self.in_names) + list(self.out_names)
        if partition_name is not None:
            all_in.append(partition_name)
        donate = tuple(range(self.n_params,
                             self.n_params + len(self.out_names)))
        out_avals_t = tuple(out_avals)
        out_names_t = tuple(self.out_names)
        all_in_t = tuple(all_in)

        def _body(*args):
            operands = list(args)
            if partition_name is not None:
                operands.append(bass2jax.partition_id_tensor())
            outs = bass2jax._bass_exec_p.bind(
                *operands, out_avals=out_avals_t, in_names=all_in_t,
                out_names=out_names_t, lowering_input_output_aliases=(),
                sim_require_finite=True, sim_require_nnan=True, nc=nc)
            return tuple(outs)

        devices = jax.devices()[:n_cores]
        mesh = Mesh(np.asarray(devices), ("core",))
        n_out = len(self.out_names)
        in_specs = (PartitionSpec("core"),) * (self.n_params + n_out)
        out_specs = (PartitionSpec("core"),) * n_out
        from jax.sharding import NamedSharding
        self.sharding = NamedSharding(mesh, PartitionSpec("core"))
        self.fn = jax.jit(
            shard_map(_body, mesh=mesh, in_specs=in_specs,
                      out_specs=out_specs, check_rep=False),
            donate_argnums=donate, keep_unused=True)

    def put_zeros(self):
        """Donatable output buffers. The kernel fully overwrites its
        outputs, so after the first call we recycle the previous call's
        device-resident outputs (already fetched to host) instead of
        shipping fresh zero buffers — no h2d RPC at all."""
        import jax
        recycled = getattr(self, "_last_out", None)
        if recycled is not None and all(not o.is_deleted() for o in recycled):
            return list(recycled)
        return [jax.device_put(np.zeros_like(z), self.sharding)
                for z in self.zero_outs]

    def __call__(self, concat_inputs, dev_zeros=None):
        """concat_inputs: name -> global array (n_cores*dim0, ...)."""
        args = [concat_inputs[n] for n in self.in_names]
        zeros = (dev_zeros if dev_zeros is not None
                 else [np.zeros_like(z) for z in self.zero_outs])
        out = self.fn(*args, *zeros)
        res = {n: np.asarray(out[i]) for i, n in enumerate(self.out_names)}
        self._last_out = list(out)
        return res


_PREP_CACHE = {}


def _get_prep_fns(N, D, C, n_cores, inv_T):
    """Two fused XLA-CPU jits: prep_x (packed xst shards, put first so
    its h2d overlaps the rest) and prep_rest (packed mask + folded
    correction pairs)."""
    key = (N, D, C, n_cores, inv_T)
    if key in _PREP_CACHE:
        return _PREP_CACHE[key]
    import jax
    import jax.numpy as jnp

    R = N // n_cores
    RH = R // 2
    NB = R // P
    E0 = float(np.exp(inv_T))

    def prep_rest(X, A, CM, L):
        """Mask+vectors FIRST (owns the nx2 einsum) so the 2.1 MB mpk
        h2d dispatches ~15 ms into the call instead of last."""
        nx2 = jnp.einsum("ij,ij->i", X, X)
        # plane-major bit-pack: byte k bit b <-> col b*(N/8)+k
        u8 = CM.astype(jnp.uint8).reshape(C, 8, N // 8)
        pk = (u8 << jnp.arange(8, dtype=jnp.uint8)[None, :, None]).sum(
            1).astype(jnp.uint8)
        mpk = pk[L]
        na2 = jnp.einsum("ij,ij->i", A, A)
        dot = jnp.einsum("ij,ij->i", X, A)
        den = jnp.maximum(jnp.sqrt(nx2) * jnp.sqrt(na2), EPS)
        p = jnp.exp(dot / den * inv_T)
        md = CM[L, jnp.arange(N)].astype(jnp.float32)
        cnum = (p - E0 * md).astype(jnp.float32)
        cden = (p - E0).astype(jnp.float32)
        cvec = jnp.stack([cnum, cden], axis=-1)              # [N, 2] f32
        cvb = jax.lax.bitcast_convert_type(
            cvec, jnp.uint8).reshape(N, 8)                   # LE bytes
        return jnp.concatenate([mpk, cvb], axis=1), nx2      # [N, N/8+8]

    def prep_x(X, nx2):
        rinv = XS / jnp.maximum(jnp.sqrt(nx2), 1e-30)
        Y = X * rinv[:, None]                       # x-hat * XS, std ~0.5
        q = jnp.clip(jnp.round(Y / QL), -8, 7) + 8.0
        v = q.astype(jnp.uint8)
        # per-core [R, D] -> [D, R]; pack column pairs (r, r+R/2)
        vt = v.reshape(n_cores, R, D).transpose(0, 2, 1)     # [8, D, R]
        pk = vt[:, :, :RH] | (vt[:, :, RH:] << 4)            # [8, D, R/2]
        return pk.reshape(n_cores * D, RH)

    fns = (jax.jit(prep_x), jax.jit(prep_rest))
    _PREP_CACHE[key] = fns
    return fns


def _prepare(inst_embed, anchor, cls_mask, labels, inv_T, n_cores,
             put=None):
    """Host marshalling. If ``put`` is given, each array is handed to it
    as soon as it's ready (async device_put overlaps later prep)."""
    import jax

    N, D = inst_embed.shape
    C = cls_mask.shape[0]
    if put is None:
        put = lambda a: np.asarray(a)
    prep_x, prep_rest = _get_prep_fns(N, D, C, n_cores, inv_T)

    X = np.ascontiguousarray(inst_embed, dtype=np.float32)
    A = np.ascontiguousarray(anchor, dtype=np.float32)
    L = np.asarray(labels)
    CM = np.ascontiguousarray(cls_mask, dtype=np.int32)
    cpu = jax.devices("cpu")[0]
    out = {}
    with jax.default_device(cpu):
        mpk, nx2 = prep_rest(X, A, CM, L)
        # device_put of a LAZY cpu array blocks on its compute; materialize
        # first so the put dispatches async and the h2d overlaps prep_x.
        mpk.block_until_ready()
        out["mpk"] = put(mpk)
        xp = prep_x(X, nx2)
        xp.block_until_ready()
    out["xp"] = put(xp)
    return out


def run(inst_embed, anchor, cls_mask, labels, temperature, n_cores=8):
    """Build+compile (cached), run on hardware, reduce. Returns loss f32."""
    from concourse.bass_interp import get_hw_module

    N, D = inst_embed.shape
    R = N // n_cores
    inv_T = float(1.0 / np.float32(temperature))
    key = (N, D, R, inv_T)
    if key not in _CACHE:
        nc = build_kernel(N, D, R, inv_T, n_cores=n_cores)
        nc.m = get_hw_module(nc.m)
        _CACHE[key] = _Runner(nc, n_cores)
    runner = _CACHE[key]

    import jax
    put = lambda a: jax.device_put(a, runner.sharding)
    dev_zeros = runner.put_zeros()
    cat = _prepare(inst_embed, anchor, cls_mask, labels, inv_T, n_cores,
                   put=put)
    res = runner(cat, dev_zeros=dev_zeros)
    vals = np.asarray(res["logq"], dtype=np.float32).reshape(-1)
    loss = -np.mean(vals.astype(np.float64))
    return np.array(loss, dtype=np.float32)


def kernel(inst_embed, anchor, cls_mask, labels, temperature):
    return run(inst_embed, anchor, cls_mask, labels, temperature)


# revision 19
# speedup vs baseline: 1.1542x; 1.1542x over previous
"""Conditional_Embedding_Contrastive_loss Trainium2 kernel (8 cores).

Full-input contract: kernel(**inputs) takes the complete tensors and
returns the scalar loss. End-to-end wall time is dominated by the axon
host->device tunnel (~60-110 MB/s effective, ~50ms sync RTT) and by
host-side marshalling, so the implementation minimizes bytes moved and
round trips:

  1. Each core receives ONLY its own int4-packed shard of the
     row-normalized embedding matrix (x-hat * 16 quantized to step 0.25,
     two columns per byte; 256 KB/core). The full operand is assembled
     on-device with a DRAM AllGather over NeuronLink and unpacked to fp8
     (the 17 quantized levels are exactly fp8-representable, so int4
     packing costs no extra precision vs the quantization itself).
  2. Row norms, the anchor cosine term p_i, and the analytic diagonal
     corrections are computed on the host (one fused XLA-CPU jit) and
     folded into a tiny per-row pair (cnum, cden):
         logq_i = ln(S_msk_i + cnum_i) - ln(S_all_i + cden_i)
     with cnum_i = p_i - exp(1/T)*m_ii, cden_i = p_i - exp(1/T), where
     S_all/S_msk are full-row sums of exp(sim/T) (resp. masked by
     cls_mask[labels_i]) including the diagonal.
  3. The 0/1 mask rows are bit-packed on the host (plane-major: byte k,
     bit b <-> column b*(N/8)+k) to 256 KB/core and unpacked on-device
     with shift+and DVE ops. The (cnum, cden) f32 pair rides as 8
     trailing bytes per mask row (read on device via AP bitcast), so the
     whole call issues only two h2d arrays + one d2h fetch.
  4. The shard_map jit is built once per process and cached; prep jit
     outputs are materialized before device_put so the h2d of each
     array overlaps the compute of the next (device_put of a lazy cpu
     array would block).

Device pipeline per core (R = N/8 = 512 rows, P = 128):
  - DRAM AllGather: xp [D, R/2] u8 -> xg [8*D, R/2].
  - int4 unpack: (b&15), (b>>4)&15 -> fp8 via TSP mult/sub (u8 in, fp8
    out) into xt_sb [128, D/128, N] fp8; own shard likewise.
  - per row-block b (4) and j-tile (1024 cols): PE fp8 matmul (8
    k-chunks, 2x512-wide) -> PSUM; ACT exp(scale=1/(T*256)) PSUM->SBUF
    with accum_out = unmasked row-sum; DVE scalar_tensor_tensor e*mask
    with accum_out = masked row-sum.
  - tail per block: two Ln on ACT, subtract, DMA out logq [NB,P,1].
Host: loss = -mean(logq).
"""

import sys

for _p in ("/opt/trn_rl_repo",):
    if _p not in sys.path:
        sys.path.insert(0, _p)

import numpy as np
import ml_dtypes

P = 128          # SBUF partitions
JW = 512         # PE moving free-dim max
EPS = 1e-8

_CACHE = {}

XS = 16.0        # pre-scale: matmul yields XS^2 * sim, folded out in the exp
QL = 0.25        # int4 quant step of (x-hat * XS); levels (v-8)*QL


def build_kernel(N, D, R, inv_T, n_cores=8, shared_cc_out=True,
                 mpsum_bufs=3, work_bufs=2, mask_bufs=2, stage_bufs=3):
    """Build the SPMD Bass program for one core owning R rows of N total."""
    import concourse.bass as bass
    import concourse.mybir as mybir
    import concourse.tile as tile
    from concourse import bacc

    f32 = mybir.dt.float32
    bf16 = mybir.dt.bfloat16
    fp8 = mybir.dt.float8e4
    u8 = mybir.dt.uint8
    exp_scale = float(inv_T / (XS * XS))
    Exp = mybir.ActivationFunctionType.Exp
    Ln = mybir.ActivationFunctionType.Ln
    mult = mybir.AluOpType.mult
    sub = mybir.AluOpType.subtract
    shr = mybir.AluOpType.logical_shift_right
    band = mybir.AluOpType.bitwise_and
    X = mybir.AxisListType.X

    KC = D // P        # contraction chunks of 128
    NB = R // P        # own row blocks
    RH = R // 2        # packed bytes per row-shard line (2 cols/byte)
    JT = min(1024, N)  # j-tile width (2 PSUM banks of fp32)
    JC = N // JT       # j tiles per row block
    NH = JT // JW      # matmuls per j-tile per k-chunk
    NPB = N // 8       # packed-mask bytes per row (one bit-plane's width)

    nc = bacc.Bacc(
        "TRN2", target_bir_lowering=False, debug=False, num_devices=n_cores)
    xp_d = nc.declare_dram_parameter("xp", [D, RH], u8, isOutput=False)
    # mask rows + 8 trailing bytes per row = (cnum, cden) f32 pair
    mpk_d = nc.declare_dram_parameter("mpk", [R, NPB + 8], u8, isOutput=False)
    out_d = nc.declare_dram_parameter("logq", [NB, P, 1], f32, isOutput=True)

    with tile.TileContext(nc) as tc:
        with (
            tc.tile_pool(name="big", bufs=1) as big,
            tc.tile_pool(name="stage", bufs=stage_bufs) as stagep,
            tc.tile_pool(name="mask", bufs=mask_bufs) as maskp,
            tc.tile_pool(name="work", bufs=work_bufs) as workp,
            tc.tile_pool(name="stats", bufs=1) as statsp,
            tc.tile_pool(name="tiny", bufs=2) as tinyp,
            tc.tile_pool(name="dram", bufs=1, space="DRAM") as dramp,
            tc.tile_pool(name="mpsum", bufs=mpsum_bufs, space="PSUM") as mpsum,
        ):
            xt_sb = big.tile([P, KC, N], fp8)
            xst_sb = big.tile([P, KC, R], fp8)
            mpk_sb = big.tile([P, NB, NPB], u8)
            cv_sb = statsp.tile([P, NB, 8], u8)
            accA = statsp.tile([P, NB, JC], f32)
            accM = statsp.tile([P, NB, JC], f32)
            logq = statsp.tile([P, NB], f32)

            xin_b = dramp.tile([D, RH], u8)
            xg_b = dramp.tile(
                [n_cores * D, RH], u8,
                addr_space="Shared" if shared_cc_out else "Local")

            # ---- collective: own packed shard -> full gathered matrix ----
            nc.sync.dma_start(xin_b[:], xp_d[:, :])
            nc.gpsimd.collective_compute(
                "AllGather", mybir.AluOpType.bypass,
                replica_groups=[list(range(n_cores))],
                ins=[xin_b.opt()], outs=[xg_b.opt()])

            # ---- input DMAs that don't depend on the collective ----
            for b in range(NB):
                nc.sync.dma_start(
                    mpk_sb[:, b, :], mpk_d[b * P:(b + 1) * P, 0:NPB])
                nc.sync.dma_start(
                    cv_sb[:, b, :], mpk_d[b * P:(b + 1) * P, NPB:NPB + 8])

            # Pre-place the combined ln+exp activation table (a table switch
            # costs ~2.7us on the scalar engine).
            ACT_SET_LN_EXP = 6  # natural_log_exp_and_others (gen3 act_info)
            nc.scalar.add_instruction(mybir.InstLoadActFuncSet(
                name=nc.get_next_instruction_name(),
                act_func_set_id=ACT_SET_LN_EXP, ins=[], outs=[]))

            def unpack4(dst_lo, dst_hi, src_u8):
                """int4 pair -> two fp8 column groups: (v-8)*QL each."""
                lo = stagep.tile([P, RH], u8, tag="lo", name="lo")
                hi = stagep.tile([P, RH], u8, tag="hi", name="hi")
                nc.vector.tensor_scalar(lo, src_u8, 15, None, op0=band)
                nc.vector.tensor_scalar(hi, src_u8, 4, 15, op0=shr, op1=band)
                # arith TSP casts u8 -> fp8: out = v*QL - 8*QL
                nc.vector.tensor_scalar(
                    dst_lo, lo, float(QL), float(8 * QL), op0=mult, op1=sub)
                nc.vector.tensor_scalar(
                    dst_hi, hi, float(QL), float(8 * QL), op0=mult, op1=sub)

            # ---- own shard unpack (param direct; overlaps collective) ----
            for c in range(KC):
                pko = stagep.tile([P, RH], u8, tag="pk", name="pko")
                nc.sync.dma_start(pko, xp_d[c * P:(c + 1) * P, :])
                unpack4(xst_sb[:, c, 0:RH], xst_sb[:, c, RH:R], pko)

            # ---- gathered shards -> SBUF (unpacked) ----
            for k in range(n_cores):
                for c in range(KC):
                    pkg = stagep.tile([P, RH], u8, tag="pk", name="pkg")
                    nc.sync.dma_start(
                        pkg, xg_b[k * D + c * P: k * D + (c + 1) * P, :])
                    unpack4(xt_sb[:, c, k * R: k * R + RH],
                            xt_sb[:, c, k * R + RH: (k + 1) * R], pkg)

            # ---- main loop ----
            for b in range(NB):
                # unpack this block's mask rows: bit-plane pl covers columns
                # [pl*NPB, (pl+1)*NPB). bitVec TSP ops can't cast dtypes, so
                # (>>pl)&1 stays u8->u8 and a mult-by-1 TSP does u8->bf16.
                m_sb = maskp.tile([P, N], bf16, tag="m", name="m_sb")
                for pl in range(8):
                    msh = maskp.tile([P, NPB], u8, tag="msh", name="msh")
                    nc.vector.tensor_scalar(
                        msh, mpk_sb[:, b, :], pl, 1, op0=shr, op1=band)
                    nc.vector.tensor_scalar_mul(
                        m_sb[:, pl * NPB:(pl + 1) * NPB], msh, 1)
                for jq in range(JC):
                    ps = mpsum.tile([P, JT], f32, tag="ps", name="ps")
                    for c in range(KC):
                        for h in range(NH):
                            nc.tensor.matmul(
                                ps[:, h * JW:(h + 1) * JW],
                                xst_sb[:, c, b * P:(b + 1) * P],
                                xt_sb[:, c, jq * JT + h * JW:
                                      jq * JT + (h + 1) * JW],
                                start=(c == 0), stop=(c == KC - 1))
                    e = workp.tile([P, JT], f32, tag="e", name="e")
                    nc.scalar.activation(
                        e, ps[:], Exp, scale=exp_scale,
                        accum_out=accA[:, b, jq:jq + 1])
                    junk = workp.tile([P, JT], f32, tag="junk", name="junk")
                    nc.vector.scalar_tensor_tensor(
                        out=junk, in0=e, scalar=1.0,
                        in1=m_sb[:, jq * JT:(jq + 1) * JT],
                        op0=mult, op1=mult,
                        accum_out=accM[:, b, jq:jq + 1])
                # tail: logq for block b
                sA = tinyp.tile([P, 1], f32, tag="sA")
                sM = tinyp.tile([P, 1], f32, tag="sM")
                nc.vector.reduce_sum(sA, accA[:, b, :], axis=X)
                nc.vector.reduce_sum(sM, accM[:, b, :], axis=X)
                num = tinyp.tile([P, 1], f32, tag="num")
                den = tinyp.tile([P, 1], f32, tag="den")
                cv = cv_sb[:, b, :].bitcast(f32)
                nc.vector.tensor_add(num, sM, cv[:, 0:1])
                nc.vector.tensor_add(den, sA, cv[:, 1:2])
                lnn = tinyp.tile([P, 1], f32, tag="lnn")
                lnd = tinyp.tile([P, 1], f32, tag="lnd")
                nc.scalar.activation(lnn, num, Ln)
                nc.scalar.activation(lnd, den, Ln)
                nc.vector.tensor_sub(logq[:, b:b + 1], lnn, lnd)
                nc.sync.dma_start(out_d[b], logq[:, b:b + 1])

    nc.compile()
    return nc


class _Runner:
    """shard_map jit built once; warm calls skip trace/lower/compile."""

    def __init__(self, nc, n_cores):
        import jax
        from jax.sharding import Mesh, PartitionSpec
        try:
            from jax.experimental.shard_map import shard_map
        except ImportError:
            from jax import shard_map
        import concourse.mybir as mybir
        from concourse import bass2jax

        bass2jax.install_neuronx_cc_hook()
        self.n_cores = n_cores
        self.in_names = []
        self.out_names = []
        out_avals = []
        self.zero_outs = []
        partition_name = (nc.partition_id_tensor.name
                          if nc.partition_id_tensor else None)
        for alloc in nc.m.functions[0].allocations:
            if not isinstance(alloc, mybir.MemoryLocationSet):
                continue
            name = alloc.memorylocations[0].name
            if alloc.kind == "ExternalInput":
                if name != partition_name:
                    self.in_names.append(name)
            elif alloc.kind == "ExternalOutput":
                shape = tuple(alloc.tensor_shape)
                dtype = mybir.dt.np(alloc.dtype)
                out_avals.append(jax.core.ShapedArray(shape, dtype))
                self.out_names.append(name)
                self.zero_outs.append(np.zeros(
                    (n_cores * shape[0],) + shape[1:], dtype))
        self.n_params = len(self.in_names)
        all_in = list(self.in_names) + list(self.out_names)
        if partition_name is not None:
            all_in.append(partition_name)
        donate = tuple(range(self.n_params,
                             self.n_params + len(self.out_names)))
        out_avals_t = tuple(out_avals)
        out_names_t = tuple(self.out_names)
        all_in_t = tuple(all_in)

        def _body(*args):
            operands = list(args)
            if partition_name is not None:
                operands.append(bass2jax.partition_id_tensor())
            outs = bass2jax._bass_exec_p.bind(
                *operands, out_avals=out_avals_t, in_names=all_in_t,
                out_names=out_names_t, lowering_input_output_aliases=(),
                sim_require_finite=True, sim_require_nnan=True, nc=nc)
            return tuple(outs)

        devices = jax.devices()[:n_cores]
        mesh = Mesh(np.asarray(devices), ("core",))
        n_out = len(self.out_names)
        in_specs = (PartitionSpec("core"),) * (self.n_params + n_out)
        out_specs = (PartitionSpec("core"),) * n_out
        from jax.sharding import NamedSharding
        self.sharding = NamedSharding(mesh, PartitionSpec("core"))
        self.fn = jax.jit(
            shard_map(_body, mesh=mesh, in_specs=in_specs,
                      out_specs=out_specs, check_rep=False),
            donate_argnums=donate, keep_unused=True)

    def put_zeros(self):
        """Donatable output buffers. The kernel fully overwrites its
        outputs, so after the first call we recycle the previous call's
        device-resident outputs (already fetched to host) instead of
        shipping fresh zero buffers — no h2d RPC at all."""
        import jax
        recycled = getattr(self, "_last_out", None)
        if recycled is not None and all(not o.is_deleted() for o in recycled):
            return list(recycled)
        return [jax.device_put(np.zeros_like(z), self.sharding)
                for z in self.zero_outs]

    def __call__(self, concat_inputs, dev_zeros=None):
        """concat_inputs: name -> global array (n_cores*dim0, ...)."""
        args = [concat_inputs[n] for n in self.in_names]
        zeros = (dev_zeros if dev_zeros is not None
                 else [np.zeros_like(z) for z in self.zero_outs])
        out = self.fn(*args, *zeros)
        res = {n: np.asarray(out[i]) for i, n in enumerate(self.out_names)}
        self._last_out = list(out)
        return res


_PREP_CACHE = {}


def _get_prep_fns(N, D, C, n_cores, inv_T):
    """Two fused XLA-CPU jits: prep_x (packed xst shards, put first so
    its h2d overlaps the rest) and prep_rest (packed mask + folded
    correction pairs)."""
    key = (N, D, C, n_cores, inv_T)
    if key in _PREP_CACHE:
        return _PREP_CACHE[key]
    import jax
    import jax.numpy as jnp

    R = N // n_cores
    RH = R // 2
    NB = R // P
    E0 = float(np.exp(inv_T))

    def prep_rest(X, A, CM, L):
        """Mask+vectors FIRST (owns the nx2 einsum) so the 2.1 MB mpk
        h2d dispatches ~15 ms into the call instead of last."""
        nx2 = jnp.einsum("ij,ij->i", X, X)
        # plane-major bit-pack: byte k bit b <-> col b*(N/8)+k
        u8 = CM.astype(jnp.uint8).reshape(C, 8, N // 8)
        pk = (u8 << jnp.arange(8, dtype=jnp.uint8)[None, :, None]).sum(
            1).astype(jnp.uint8)
        mpk = pk[L]
        na2 = jnp.einsum("ij,ij->i", A, A)
        dot = jnp.einsum("ij,ij->i", X, A)
        den = jnp.maximum(jnp.sqrt(nx2) * jnp.sqrt(na2), EPS)
        p = jnp.exp(dot / den * inv_T)
        md = CM[L, jnp.arange(N)].astype(jnp.float32)
        cnum = (p - E0 * md).astype(jnp.float32)
        cden = (p - E0).astype(jnp.float32)
        cvec = jnp.stack([cnum, cden], axis=-1)              # [N, 2] f32
        cvb = jax.lax.bitcast_convert_type(
            cvec, jnp.uint8).reshape(N, 8)                   # LE bytes
        return jnp.concatenate([mpk, cvb], axis=1), nx2      # [N, N/8+8]

    def prep_x(X, nx2):
        # quant scale folded into the per-row normalizer: one fused
        # multiply+rint+clip+add pass over X instead of two multiplies
        rq = (XS / QL) / jnp.maximum(jnp.sqrt(nx2), 1e-30)
        q = jnp.clip(jnp.rint(X * rq[:, None]), -8, 7) + 8.0
        v = q.astype(jnp.uint8)
        # per-core [R, D] -> [D, R]; pack column pairs (r, r+R/2)
        vt = v.reshape(n_cores, R, D).transpose(0, 2, 1)     # [8, D, R]
        pk = vt[:, :, :RH] | (vt[:, :, RH:] << 4)            # [8, D, R/2]
        return pk.reshape(n_cores * D, RH)

    fns = (jax.jit(prep_x), jax.jit(prep_rest))
    _PREP_CACHE[key] = fns
    return fns


def _prepare(inst_embed, anchor, cls_mask, labels, inv_T, n_cores,
             put=None):
    """Host marshalling. If ``put`` is given, each array is handed to it
    as soon as it's ready (async device_put overlaps later prep)."""
    import jax

    N, D = inst_embed.shape
    C = cls_mask.shape[0]
    if put is None:
        put = lambda a: np.asarray(a)
    prep_x, prep_rest = _get_prep_fns(N, D, C, n_cores, inv_T)

    X = np.ascontiguousarray(inst_embed, dtype=np.float32)
    A = np.ascontiguousarray(anchor, dtype=np.float32)
    L = np.asarray(labels)
    CM = np.ascontiguousarray(cls_mask, dtype=np.int32)
    cpu = jax.devices("cpu")[0]
    out = {}
    with jax.default_device(cpu):
        mpk, nx2 = prep_rest(X, A, CM, L)
        # device_put of a LAZY cpu array blocks on its compute; materialize
        # first so the put dispatches async and the h2d overlaps prep_x.
        mpk.block_until_ready()
        out["mpk"] = put(mpk)
        xp = prep_x(X, nx2)
        xp.block_until_ready()
    out["xp"] = put(xp)
    return out


def run(inst_embed, anchor, cls_mask, labels, temperature, n_cores=8):
    """Build+compile (cached), run on hardware, reduce. Returns loss f32."""
    from concourse.bass_interp import get_hw_module

    N, D = inst_embed.shape
    R = N // n_cores
    inv_T = float(1.0 / np.float32(temperature))
    key = (N, D, R, inv_T)
    if key not in _CACHE:
        nc = build_kernel(N, D, R, inv_T, n_cores=n_cores)
        nc.m = get_hw_module(nc.m)
        _CACHE[key] = _Runner(nc, n_cores)
    runner = _CACHE[key]

    import jax
    put = lambda a: jax.device_put(a, runner.sharding)
    dev_zeros = runner.put_zeros()
    cat = _prepare(inst_embed, anchor, cls_mask, labels, inv_T, n_cores,
                   put=put)
    res = runner(cat, dev_zeros=dev_zeros)
    vals = np.asarray(res["logq"], dtype=np.float32).reshape(-1)
    loss = -np.mean(vals.astype(np.float64))
    return np.array(loss, dtype=np.float32)


def kernel(inst_embed, anchor, cls_mask, labels, temperature):
    return run(inst_embed, anchor, cls_mask, labels, temperature)
